# revision 31
# baseline (speedup 1.0000x reference)
"""DeepseekV3 MLA flash-attention prefill kernel for 8 Trainium2 NeuronCores.

Sharding (SPMD, one program for all 8 cores):
  Stage A (sequence-parallel): core c owns 256 seq rows. Inputs arrive as a
    dependency-chained sequence of large packed DMAs (x || wa_kv, then the
    wa_q quarters, then stage-B weights) so early tiles are never delayed by
    later transfers interleaving on the same queue. Each weight wave
    accumulates into bank-exclusive PSUM groups. The kv AllGather fires right
    at the initial-barrier horizon; the q AllGather carries RAW (unnormalized)
    qa plus the rms scale row, applied post-projection in stage B.
  Stage B (head-parallel): core c owns heads {2c, 2c+1}. K^T/V from the kv
    gather. Causal attention in (k, q) layout, no max-subtraction,
    fully-masked k-blocks skipped, diagonal blocks masked by a vector
    mask-add (softmax scale pre-folded into Wqb host-side).
  Output: per-panel partial Wo products (only this core's 2 head-rows of Wo)
    are exchanged with one AllToAll per 512-row panel and reduced on-core in
    f32; earlier panels' exchanges hide under later (heavier) panels'
    attention. The last panel's exchange is split into two hid-halves so its
    first half's reduction overlaps the second half's transfer.
"""

import sys

if '/opt/trn_rl_repo' not in sys.path:
    sys.path.insert(0, '/opt/trn_rl_repo')

import numpy as np
import ml_dtypes

import concourse.bass as bass
import concourse.mybir as mybir
import concourse.tile as tile
from concourse import bacc
from concourse.bass_utils import run_bass_kernel_spmd

f32 = mybir.dt.float32
f32r = mybir.dt.float32r
bf16 = mybir.dt.bfloat16
i32 = mybir.dt.int32
AF = mybir.ActivationFunctionType
ALU = mybir.AluOpType

NC_ = 8            # cores
S = 2048           # sequence
HID = 2048
QLR = 1536         # q lora rank
KVLR = 512         # kv lora rank
ROPE = 64
NOPE = 128
VD = 128
NH = 16
HPC = NH // NC_    # heads per core = 2
SL = S // NC_      # rows per core = 256
PANEL = 512        # q panel width
NPANEL = S // PANEL
NKB = S // 128     # 16 k blocks
QCH = QLR // 128   # 12
QHALF = QCH // 2   # 6
KCH = KVLR // 128  # 4
HCH = HID // 128   # 16
KVW = KVLR + ROPE  # 576 = kv wave width
SHARD = PANEL // NC_  # 64 rows per core per panel
THETA = 10000.0
SM_SCALE = float((NOPE + ROPE) ** -0.5)
PI = float(np.pi)
NEG = -1e30

DT = bf16

_CACHE = {}


def _range_reduce_sin(nc, pool, src_ap, P, W, bias, name, res_pool=None, res_dt=f32, tagw=""):
    """sin(src + bias) with range reduction to [-pi, pi]. src may be PSUM."""
    t0 = pool.tile([P, W], f32, name=f"{name}_t0", tag=f"rr0{tagw}", bufs=1)
    ti = pool.tile([P, W], i32, name=f"{name}_ti", tag=f"rr1{tagw}", bufs=1)
    tf = pool.tile([P, W], f32, name=f"{name}_tf", tag=f"rr2{tagw}", bufs=1)
    arg = pool.tile([P, W], f32, name=f"{name}_arg", tag=f"rr3{tagw}", bufs=1)
    res = (res_pool or pool).tile([P, W], res_dt, name=f"{name}_sin", tag=f"res_{name}", bufs=1)
    nc.vector.tensor_scalar(out=t0[:], in0=src_ap, scalar1=bias, scalar2=None, op0=ALU.add)
    nc.vector.tensor_scalar(out=tf[:], in0=t0[:], scalar1=1.0 / (2 * PI), scalar2=None, op0=ALU.mult)
    nc.vector.tensor_copy(ti[:], tf[:])
    nc.vector.tensor_copy(tf[:], ti[:])
    nc.vector.scalar_tensor_tensor(out=arg[:], in0=tf[:], scalar=-2 * PI, in1=t0[:], op0=ALU.mult, op1=ALU.add)
    nc.scalar.activation(res[:], arg[:], AF.Sin)
    return res


def build_program(dt):
    nc = bacc.Bacc("TRN2", target_bir_lowering=False, debug=False, num_devices=NC_)

    def din(name, shape):
        return nc.dram_tensor(name, shape, dt, kind="ExternalInput")

    # ---- external I/O (per-core data, packed for large-row DMAs) ----
    x_p = din("x_p", [128, HCH * SL])            # hc-major packed X^T
    pos = nc.dram_tensor("pos", [1, SL], f32, kind="ExternalInput")
    pos_all = nc.dram_tensor("pos_all", [1, S], f32, kind="ExternalInput")
    wakv_p = din("wakv_p", [128, HCH * KVW])     # [Wkva(kv)|Wkva(pe,deint)] per hc
    waq_p = din("waq_p", [128, HCH * QLR])       # Wqa per hc
    wqb_p = din("wqb_p", [128, QCH * HPC * 256])  # [nope|pe_d|rot]*SM per head, per l
    wkk_p = din("wkk_p", [128, KCH * HPC * NOPE])
    wkv_p = din("wkv_p", [128, KCH * HPC * VD])
    wo_p = din("wo_p", [128, HPC * HID])         # Wo rows for this core's heads
    mask_in = din("mask", [128, 4 * PANEL])      # diag masks j=0..3 (0 / -1e30)
    ones_col = din("ones_col", [128, 1])
    ones_row = nc.dram_tensor("ones_row", [1, 128], f32, kind="ExternalInput")
    invf_col = nc.dram_tensor("invf_col", [ROPE, 1], f32, kind="ExternalInput")
    out_loc = nc.dram_tensor("out_loc", [NPANEL * SHARD, HID], f32, kind="ExternalOutput")

    QROWS = QCH * 128 + 1  # 12 raw chunks + rms scale row

    with tile.TileContext(nc) as tc:
        with tc.tile_pool(name="dram", bufs=1, space="DRAM") as dpool, \
             tc.tile_pool(name="persist", bufs=1) as rp:
            ag_in_kv = dpool.tile([KVW, SL], dt)
            ag_out_kv = dpool.tile([NC_ * KVW, SL], dt, addr_space="Shared")
            ag_in_q = dpool.tile([QROWS, SL], dt)
            ag_out_q = dpool.tile([NC_ * QROWS, SL], dt, addr_space="Shared")
            a2a_in = {p: dpool.tile([PANEL, HID], dt, name=f"a2a_in{p}")
                      for p in range(NPANEL - 1)}
            a2a_out = {p: dpool.tile([PANEL, HID], dt, name=f"a2a_out{p}")
                       for p in range(NPANEL - 1)}
            # last panel exchanged in two hid-halves
            a2a_lin = [dpool.tile([PANEL, HID // 2], dt, name=f"a2a_lin{i}") for i in range(2)]
            a2a_lout = [dpool.tile([PANEL, HID // 2], dt, name=f"a2a_lout{i}") for i in range(2)]

            # ---- constants ----
            ocol = rp.tile([128, 1], dt)
            orow = rp.tile([1, 128], f32r)
            orow_bf = rp.tile([1, 128], dt)
            invc_t = rp.tile([ROPE, 1], f32)
            nc.sync.dma_start(out=ocol[:], in_=ones_col[:])
            nc.sync.dma_start(out=orow[:], in_=ones_row[:].bitcast(f32r))
            nc.sync.dma_start(out=invc_t[:], in_=invf_col[:])
            nc.vector.tensor_copy(orow_bf[:], orow[:].bitcast(f32))

            # stage B weight tiles (DMAs chained below)
            mask_sb = rp.tile([128, 4 * PANEL], dt, name="mask_sb")
            wqb_all = rp.tile([128, QCH * HPC * 256], dt, name="wqb_all")
            wkk_all = rp.tile([128, KCH * HPC * NOPE], dt, name="wkk_all")
            wkv_all = rp.tile([128, KCH * HPC * VD], dt, name="wkv_all")
            wo_all = rp.tile([128, HPC * HID], dt, name="wo_all")

            def wqb_t(l):
                return wqb_all[:, 512 * l:512 * (l + 1)]

            def wkk_t(l):
                return wkk_all[:, 256 * l:256 * (l + 1)]

            def wkv_t(l):
                return wkv_all[:, 256 * l:256 * (l + 1)]

            def wo_sb(h):
                return wo_all[:, HID * h:HID * (h + 1)]

            sin_all = None
            cos_all = None

            # ================= Stage A =================
            with tc.tile_pool(name="sa_in", bufs=1) as sap, \
                 tc.tile_pool(name="sa_tmp", bufs=2) as tp, \
                 tc.tile_pool(name="sa_ps", bufs=6, space="PSUM") as accp, \
                 tc.tile_pool(name="sa_ps1", bufs=1, space="PSUM") as pp1:

                # x and wa_kv in parallel; everything later is chained behind
                # them with 1-element anchor copies on the (otherwise idle)
                # gpsimd queue so one HWDGE queue never interleaves a later
                # transfer with an earlier, urgent one.
                x_all = sap.tile([128, HCH * SL], dt, name="x_all")
                nc.sync.dma_start(out=x_all[:], in_=x_p[:])
                wakv_all = sap.tile([128, HCH * KVW], dt, name="wakv_all")
                nc.sync.dma_start(out=wakv_all[:], in_=wakv_p[:])
                waq_all = sap.tile([128, HCH * QLR], dt, name="waq_all")
                NQQ = 4
                wq = HCH * QLR // NQQ
                prev_anchor = wakv_all
                for qq in range(NQQ):
                    dst = waq_all[:, wq * qq:wq * (qq + 1)]
                    nc.gpsimd.tensor_copy(waq_all[0:1, wq * qq:wq * qq + 1],
                                          prev_anchor[0:1, 0:1])
                    nc.sync.dma_start(out=dst, in_=waq_p[:, wq * qq:wq * (qq + 1)])
                    prev_anchor = waq_all[:, wq * qq:wq * (qq + 1)]

                def chain_weight_dmas(anchor_tile):
                    for wtile, wsrc in ((wkk_all, wkk_p), (wkv_all, wkv_p), (wqb_all, wqb_p),
                                        (mask_sb, mask_in), (wo_all, wo_p)):
                        nc.gpsimd.tensor_copy(wtile[0:1, 0:1], anchor_tile[0:1, 0:1])
                        nc.sync.dma_start(out=wtile[:], in_=wsrc[:])

                def xt(hc):
                    return x_all[:, SL * hc:SL * (hc + 1)]

                pos_all_t = tp.tile([1, S], f32r, name="pos_all_t", tag="posa", bufs=1)
                pos_t = tp.tile([1, SL], f32r, name="pos_t", tag="poso", bufs=1)
                nc.sync.dma_start(out=pos_all_t[:], in_=pos_all[:].bitcast(f32r))
                nc.sync.dma_start(out=pos_t[:], in_=pos[:].bitcast(f32r))
                emb_all = tp.tile([ROPE, S], f32, name="emb_all", tag="emba", bufs=1)

                # rope angle tables via K=1 outer products (one PSUM bank per
                # accumulation group -- matmul start zeroes a whole bank)
                for j in range(S // SL):
                    tb = accp.tile([128, SL], f32, name=f"tb_all{j}", tag="acc", bufs=6)
                    nc.tensor.matmul(tb[0:ROPE, :], orow[0:1, 0:ROPE],
                                     pos_all_t[:, SL * j:SL * (j + 1)], start=True, stop=True)
                    nc.vector.tensor_scalar(out=emb_all[:, SL * j:SL * (j + 1)],
                                            in0=tb[0:ROPE, :], scalar1=invc_t[:],
                                            scalar2=None, op0=ALU.mult)
                tb_own = accp.tile([128, SL], f32, name="tb_own", tag="acc", bufs=6)
                nc.tensor.matmul(tb_own[0:ROPE, 0:SL], orow[0:1, 0:ROPE], pos_t[:],
                                 start=True, stop=True)
                emb_own = tp.tile([ROPE, SL], f32, name="emb_own", tag="emb_own", bufs=1)
                nc.vector.tensor_scalar(out=emb_own[:], in0=tb_own[0:ROPE, 0:SL],
                                        scalar1=invc_t[:], scalar2=None, op0=ALU.mult)

                sin_all = _range_reduce_sin(nc, tp, emb_all[:], ROPE, S, 0.0, "sa",
                                            res_pool=rp, res_dt=dt, tagw="w")
                cos_all = _range_reduce_sin(nc, tp, emb_all[:], ROPE, S, PI / 2, "ca",
                                            res_pool=rp, res_dt=dt, tagw="w")
                sin_own = _range_reduce_sin(nc, tp, emb_own[:], ROPE, SL, 0.0, "so")
                cos_own = _range_reduce_sin(nc, tp, emb_own[:], ROPE, SL, PI / 2, "co")

                # ---- kv wave: chunks c0..c3 + pe accumulate over all hc ----
                acc_kv = [accp.tile([128, SL], f32, name=f"acc_kv{c}", tag="acc", bufs=6)
                          for c in range(KCH)]
                acc_pe = accp.tile([128, SL], f32, name="acc_pe", tag="acc", bufs=6)
                for hc in range(HCH):
                    st = (hc == 0)
                    sp = (hc == HCH - 1)
                    for c in range(KCH):
                        nc.tensor.matmul(acc_kv[c][:],
                                         wakv_all[:, KVW * hc + 128 * c:KVW * hc + 128 * (c + 1)],
                                         xt(hc), start=st, stop=sp)
                    nc.tensor.matmul(acc_pe[0:ROPE, :],
                                     wakv_all[:, KVW * hc + KVLR:KVW * hc + KVW],
                                     xt(hc), start=st, stop=sp)

                # kv ssq + rms scale
                ssq_kv = pp1.tile([1, SL], f32, name="ssq_kv", tag="ssq", bufs=1)
                sqs = []
                for c in range(KCH):
                    sq = tp.tile([128, SL], dt, name=f"sqk{c}", tag="sq", bufs=4)
                    nc.scalar.activation(sq[:], acc_kv[c][:], AF.Square)
                    sqs.append(sq)
                for c in range(KCH):
                    nc.tensor.matmul(ssq_kv[:], ocol[:], sqs[c][:],
                                     start=(c == 0), stop=(c == KCH - 1))
                ms_kv = tp.tile([1, SL], f32, name="ms_kv", tag="ms", bufs=2)
                nc.scalar.activation(ms_kv[:], ssq_kv[:], AF.Sqrt, scale=1.0 / KVLR)
                rkv = tp.tile([1, SL], f32, name="rkv", tag="rr", bufs=2)
                nc.vector.reciprocal_approx_fast(out=rkv[:], in_=ms_kv[:])
                rkvr = tp.tile([1, SL], f32r, name="rkvr", tag="rrr", bufs=2)
                with nc.allow_low_precision(reason="f32r rounding of rms scale"):
                    nc.vector.tensor_copy(rkvr[:], rkv[:])
                bc_kv = pp1.tile([128, SL], f32, name="bc_kv", tag="bc", bufs=1)
                nc.tensor.matmul(bc_kv[:], orow[:], rkvr[:], start=True, stop=True)
                bckv_sb = tp.tile([128, SL], f32, name="bckv_sb", tag="bc_sb", bufs=2)
                nc.scalar.activation(bckv_sb[:], bc_kv[:], AF.Copy)

                # k_pe rope
                krot = tp.tile([ROPE, SL], f32, name="krot", tag="krot", bufs=1)
                nc.vector.tensor_scalar(out=krot[0:32, :], in0=acc_pe[32:64, :],
                                        scalar1=-1.0, scalar2=None, op0=ALU.mult)
                nc.vector.tensor_copy(krot[32:64, :], acc_pe[0:32, :])
                kro = tp.tile([ROPE, SL], f32, name="kro", tag="kro", bufs=1)
                nc.vector.tensor_mul(kro[:], acc_pe[0:ROPE, :], cos_own[:])
                krs = tp.tile([ROPE, SL], f32, name="krs", tag="krs", bufs=1)
                nc.vector.tensor_mul(krs[:], krot[:], sin_own[:])
                kfin = tp.tile([ROPE, SL], dt, name="kfin", tag="kfin", bufs=1)
                nc.vector.tensor_add(kfin[:], kro[:], krs[:])
                nc.scalar.dma_start(out=ag_in_kv[KVLR:KVLR + ROPE, :], in_=kfin[:])

                last_sck = None
                for c in range(KCH):
                    sc = tp.tile([128, SL], dt, name=f"sck{c}", tag="sc", bufs=4)
                    nc.vector.tensor_mul(sc[:], acc_kv[c][:], bckv_sb[:])
                    nc.scalar.dma_start(out=ag_in_kv[128 * c:128 * (c + 1), :], in_=sc[:])
                    last_sck = sc

                nc.gpsimd.collective_compute(
                    "AllGather", ALU.bypass,
                    replica_groups=[list(range(NC_))],
                    ins=[ag_in_kv[:]], outs=[ag_out_kv[:]],
                )
                chain_weight_dmas(last_sck)

                # ---- q wave in two PSUM halves of 6 chunks, gathered RAW ----
                ssq_q = pp1.tile([1, SL], f32, name="ssq_q", tag="ssq", bufs=1)
                sqq = []
                acc_q1 = [accp.tile([128, SL], f32, name=f"acc_q1_{c}", tag="acc", bufs=6)
                          for c in range(QHALF)]
                for hc in range(HCH):
                    st = (hc == 0)
                    sp = (hc == HCH - 1)
                    for c in range(QHALF):
                        nc.tensor.matmul(acc_q1[c][:],
                                         waq_all[:, QLR * hc + 128 * c:QLR * hc + 128 * (c + 1)],
                                         xt(hc), start=st, stop=sp)
                for c in range(QHALF):
                    raw = tp.tile([128, SL], dt, name=f"rawqa{c}", tag="sc", bufs=4)
                    nc.vector.tensor_copy(raw[:], acc_q1[c][:])
                    nc.scalar.dma_start(out=ag_in_q[128 * c:128 * (c + 1), :], in_=raw[:])
                    sq = tp.tile([128, SL], dt, name=f"sqq{c}", tag="sq", bufs=4)
                    nc.scalar.activation(sq[:], acc_q1[c][:], AF.Square)
                    sqq.append(sq)
                for c in range(QHALF):
                    nc.tensor.matmul(ssq_q[:], ocol[:], sqq[c][:],
                                     start=(c == 0), stop=False)

                acc_q2 = [accp.tile([128, SL], f32, name=f"acc_q2_{c}", tag="acc", bufs=6)
                          for c in range(QHALF)]
                for hc in range(HCH):
                    st = (hc == 0)
                    sp = (hc == HCH - 1)
                    for c in range(QHALF):
                        cc = c + QHALF
                        nc.tensor.matmul(acc_q2[c][:],
                                         waq_all[:, QLR * hc + 128 * cc:QLR * hc + 128 * (cc + 1)],
                                         xt(hc), start=st, stop=sp)
                for c in range(QHALF):
                    raw = tp.tile([128, SL], dt, name=f"rawqb{c}", tag="sc", bufs=4)
                    nc.vector.tensor_copy(raw[:], acc_q2[c][:])
                    nc.scalar.dma_start(out=ag_in_q[128 * (c + QHALF):128 * (c + QHALF + 1), :],
                                        in_=raw[:])
                    sq = tp.tile([128, SL], dt, name=f"sqq{c + QHALF}", tag="sq", bufs=4)
                    nc.scalar.activation(sq[:], acc_q2[c][:], AF.Square)
                    sqq.append(sq)
                for c in range(QHALF):
                    nc.tensor.matmul(ssq_q[:], ocol[:], sqq[c + QHALF][:],
                                     start=False, stop=(c == QHALF - 1))
                ms_q = tp.tile([1, SL], f32, name="ms_q", tag="ms", bufs=2)
                nc.scalar.activation(ms_q[:], ssq_q[:], AF.Sqrt, scale=1.0 / QLR)
                rq = tp.tile([1, SL], f32, name="rq", tag="rr", bufs=2)
                nc.vector.reciprocal_approx_fast(out=rq[:], in_=ms_q[:])
                r_bf = tp.tile([1, SL], dt, name="r_bf", tag="rbf", bufs=1)
                nc.vector.tensor_copy(r_bf[:], rq[:])
                nc.scalar.dma_start(out=ag_in_q[QCH * 128:QCH * 128 + 1, :], in_=r_bf[:])
                nc.gpsimd.collective_compute(
                    "AllGather", ALU.bypass,
                    replica_groups=[list(range(NC_))],
                    ins=[ag_in_q[:]], outs=[ag_out_q[:]],
                )

            agkv_r = ag_out_kv.rearrange("(r c) q -> r c q", r=NC_)
            agq_r = ag_out_q.rearrange("(r c) q -> r c q", r=NC_)

            # ================= Stage B =================
            with tc.tile_pool(name="sb_res", bufs=1) as sbp, \
                 tc.tile_pool(name="sb_tmp", bufs=2) as tp, \
                 tc.tile_pool(name="sb_qa", bufs=2) as qap, \
                 tc.tile_pool(name="sb_pt", bufs=4) as ptp, \
                 tc.tile_pool(name="sb_mm", bufs=2, space="PSUM") as pmm, \
                 tc.tile_pool(name="sb_at", bufs=2, space="PSUM") as pat, \
                 tc.tile_pool(name="sb_ps1", bufs=1, space="PSUM") as pp1:

                # K^T and V (both heads)
                kpe_g = sbp.tile([ROPE, S], dt, name="kpe_g")
                for r in range(NC_):
                    nc.sync.dma_start(out=kpe_g[:, SL * r:SL * (r + 1)],
                                      in_=agkv_r[r, KVLR:KVLR + ROPE, :])
                kT = [sbp.tile([128, S], dt, name=f"kT{h}") for h in range(HPC)]
                v_t = [sbp.tile([128, HPC * VD], dt, name=f"v_t{kb}") for kb in range(NKB)]
                with tc.tile_pool(name="sb_ckv", bufs=1) as ckvp:
                    ckv_g = []
                    for j in range(KCH):
                        t = ckvp.tile([128, S], dt, name=f"ckv_g{j}")
                        for r in range(NC_):
                            nc.sync.dma_start(out=t[:, SL * r:SL * (r + 1)],
                                              in_=agkv_r[r, 128 * j:128 * (j + 1), :])
                        ckv_g.append(t)
                    ei = 0
                    for h in range(HPC):
                        for kc in range(S // 512):
                            ps = pmm.tile([128, 512], f32, name=f"kt_ps{h}_{kc}", tag="mm", bufs=3)
                            for l in range(KCH):
                                nc.tensor.matmul(ps[:], wkk_t(l)[:, NOPE * h:NOPE * (h + 1)],
                                                 ckv_g[l][:, 512 * kc:512 * (kc + 1)],
                                                 start=(l == 0), stop=(l == KCH - 1))
                            if ei % 2 == 0:
                                nc.vector.tensor_copy(kT[h][:, 512 * kc:512 * (kc + 1)], ps[:])
                            else:
                                nc.scalar.activation(kT[h][:, 512 * kc:512 * (kc + 1)], ps[:], AF.Copy)
                            ei += 1
                    for kb in range(NKB):
                        ps = pmm.tile([128, HPC * VD], f32, name=f"v_ps{kb}", tag="mm", bufs=3)
                        for l in range(KCH):
                            nc.tensor.matmul(ps[:], ckv_g[l][:, 128 * kb:128 * (kb + 1)],
                                             wkv_t(l), start=(l == 0), stop=(l == KCH - 1))
                        if kb % 2 == 0:
                            nc.vector.tensor_copy(v_t[kb][:], ps[:])
                        else:
                            nc.scalar.activation(v_t[kb][:], ps[:], AF.Copy)

                # ---- q projections (single pass, rms scale applied here) ----
                qa_p = {}
                for p in range(NPANEL - 1, -1, -1):   # processing order
                    for l in range(QCH):
                        t = qap.tile([128, PANEL], dt, name=f"qa_p{p}_{l}", tag="qaa", bufs=16)
                        for r in range(2):
                            nc.sync.dma_start(out=t[:, SL * r:SL * (r + 1)],
                                              in_=agq_r[2 * p + r, 128 * l:128 * (l + 1), :])
                        qa_p[(p, l)] = t
                qn_sb = {}
                qp_sb = {}

                def qproj_panel(p):
                    qs = slice(PANEL * p, PANEL * (p + 1))
                    rrow = tp.tile([1, PANEL], dt, name=f"rrow{p}", tag="rrow", bufs=2)
                    for r in range(2):
                        nc.sync.dma_start(out=rrow[0:1, SL * r:SL * (r + 1)],
                                          in_=agq_r[2 * p + r, QCH * 128:QCH * 128 + 1, :])
                    rbc = pp1.tile([128, PANEL], f32, name=f"rbc{p}", tag="bcb", bufs=1)
                    nc.tensor.matmul(rbc[:], orow_bf[:], rrow[:], start=True, stop=True)
                    rbc_sb = tp.tile([128, PANEL], f32, name=f"rbc_sb{p}", tag="rbc_sb", bufs=2)
                    nc.scalar.activation(rbc_sb[:], rbc[:], AF.Copy)
                    for h in range(HPC):
                        hcol = 256 * h
                        ps_qn = pmm.tile([128, PANEL], f32, name=f"qn_ps{h}_{p}", tag="mm", bufs=3)
                        for l in range(QCH):
                            nc.tensor.matmul(ps_qn[:], wqb_t(l)[:, hcol:hcol + NOPE],
                                             qa_p[(p, l)][:], start=(l == 0), stop=(l == QCH - 1))
                        ps_qr = pmm.tile([128, PANEL], f32, name=f"qr_ps{h}_{p}", tag="mm", bufs=3)
                        for l in range(QCH):
                            nc.tensor.matmul(ps_qr[:], wqb_t(l)[:, hcol + NOPE:hcol + 256],
                                             qa_p[(p, l)][:], start=(l == 0), stop=(l == QCH - 1))
                        qn = sbp.tile([128, PANEL], dt, name=f"qn_sb{h}_{p}")
                        nc.vector.tensor_mul(qn[:], ps_qn[:], rbc_sb[:])
                        qn_sb[(h, p)] = qn
                        qt1 = tp.tile([ROPE, PANEL], f32, name=f"qt1_{h}_{p}", tag="qt1", bufs=2)
                        nc.vector.tensor_mul(qt1[:], ps_qr[0:ROPE, :], cos_all[:, qs])
                        qt2 = tp.tile([ROPE, PANEL], f32, name=f"qt2_{h}_{p}", tag="qt2", bufs=2)
                        nc.vector.tensor_mul(qt2[:], ps_qr[ROPE:2 * ROPE, :], sin_all[:, qs])
                        qpp = tp.tile([ROPE, PANEL], f32, name=f"qpp{h}_{p}", tag="qpp", bufs=2)
                        nc.vector.tensor_add(qpp[:], qt1[:], qt2[:])
                        qp = sbp.tile([ROPE, PANEL], dt, name=f"qp_sb{h}_{p}")
                        nc.vector.tensor_mul(qp[:], qpp[:], rbc_sb[0:ROPE, :])
                        qp_sb[(h, p)] = qp

                # ---- attention + per-panel Wo partials + AllToAll + reduce ----
                # heaviest panel first so its exchange hides under later panels
                PLIST = list(range(NPANEL - 1, -1, -1))
                with tc.tile_pool(name="sb_red", bufs=1) as redp:
                    def reduce_rows(srcs, dst_rows, tagsuf):
                        """f32 chain-reduce NC_ bf16 blocks, store to out_loc."""
                        acc = None
                        for r in range(1, NC_):
                            nxt = redp.tile(srcs[r].shape, f32, name=f"racc{tagsuf}_{r}",
                                            tag=f"racc{srcs[r].shape[1]}", bufs=2)
                            if acc is None:
                                nc.vector.tensor_add(nxt[:], srcs[0][:], srcs[1][:])
                            else:
                                nc.vector.tensor_add(nxt[:], acc[:], srcs[r][:])
                            acc = nxt
                        nc.sync.dma_start(out=dst_rows, in_=acc[:])

                    def reduce_panel(p):
                        rbs = []
                        for r in range(NC_):
                            t = redp.tile([SHARD, HID], dt, name=f"rb{p}_{r}", tag="rb", bufs=4)
                            nc.sync.dma_start(out=t[:], in_=a2a_out[p][SHARD * r:SHARD * (r + 1), :])
                            rbs.append(t)
                        reduce_rows(rbs, out_loc[SHARD * p:SHARD * (p + 1), :], f"p{p}")

                    held_at = {}
                    HELD = PLIST[0]
                    for i, p in enumerate(PLIST):
                        qproj_panel(p)
                        nkb = 4 * (p + 1)
                        at_ps = {}
                        ps_at = {h: pat.tile([128, PANEL], f32, name=f"at_ps{h}_{p}",
                                             tag="at", bufs=2) for h in range(HPC)}
                        ps_sum = {h: pp1.tile([1, PANEL], f32, name=f"sum_ps{h}_{p}",
                                              tag="sum", bufs=2) for h in range(HPC)}
                        pts = {h: {} for h in range(HPC)}

                        def consume(h, kb, nkb=nkb, ps_at=ps_at, ps_sum=ps_sum, pts=pts):
                            nc.tensor.matmul(ps_at[h][:], v_t[kb][:, VD * h:VD * (h + 1)],
                                             pts[h][kb][:],
                                             start=(kb == 0), stop=(kb == nkb - 1))
                            nc.tensor.matmul(ps_sum[h][:], ocol[:], pts[h][kb][:],
                                             start=(kb == 0), stop=(kb == nkb - 1))

                        # both heads interleaved per k-block: one head's exp/mask
                        # latency hides under the other head's matmuls
                        for kb in range(nkb):
                            for h in range(HPC):
                                ps_sc = pmm.tile([128, PANEL], f32, name=f"sc_ps{h}_{p}_{kb}",
                                                 tag="mm", bufs=3)
                                nc.tensor.matmul(ps_sc[:], kT[h][:, 128 * kb:128 * (kb + 1)],
                                                 qn_sb[(h, p)][:], start=True, stop=False)
                                nc.tensor.matmul(ps_sc[:], kpe_g[:, 128 * kb:128 * (kb + 1)],
                                                 qp_sb[(h, p)][:], start=False, stop=True)
                                pt = ptp.tile([128, PANEL], dt, name=f"pt{h}_{p}_{kb}",
                                              tag="pt", bufs=6)
                                if kb >= 4 * p:
                                    j = kb - 4 * p
                                    msc = tp.tile([128, PANEL], f32, name=f"msc{h}_{p}_{kb}",
                                                  tag="msc", bufs=3)
                                    nc.vector.tensor_add(msc[:], ps_sc[:],
                                                         mask_sb[:, PANEL * j:PANEL * (j + 1)])
                                    nc.scalar.activation(pt[:], msc[:], AF.Exp)
                                else:
                                    nc.scalar.activation(pt[:], ps_sc[:], AF.Exp)
                                pts[h][kb] = pt
                                if kb > 0:
                                    consume(h, kb - 1)
                        for h in range(HPC):
                            consume(h, nkb - 1)
                        for h in range(HPC):
                            rec = tp.tile([1, PANEL], f32, name=f"rec{h}_{p}", tag="rec", bufs=2)
                            nc.vector.reciprocal_approx_fast(out=rec[:], in_=ps_sum[h][:])
                            recr = tp.tile([1, PANEL], f32r, name=f"recr{h}_{p}", tag="recr", bufs=2)
                            with nc.allow_low_precision(reason="f32r rounding of softmax recip"):
                                nc.vector.tensor_copy(recr[:], rec[:])
                            bc = pp1.tile([128, PANEL], f32, name=f"bc_ps{h}_{p}", tag="bcb", bufs=1)
                            nc.tensor.matmul(bc[:], orow[:], recr[:], start=True, stop=True)
                            bc_sb = tp.tile([128, PANEL], f32, name=f"bc_sb{h}_{p}", tag="bc_sb", bufs=2)
                            nc.scalar.activation(bc_sb[:], bc[:], AF.Copy)
                            at_p = tp.tile([128, PANEL], dt, name=f"at_p{h}_{p}", tag="at_p", bufs=5)
                            nc.vector.tensor_mul(at_p[:], ps_at[h][:], bc_sb[:])
                            at_ps[h] = at_p

                        if p == HELD:
                            held_at = at_ps
                            continue

                        # Wo partial, full hid width, one store per seq block
                        for sb in range(4):
                            ev = tp.tile([128, HID], dt, name=f"woev{p}_{sb}", tag="woev", bufs=2)
                            for n in range(4):
                                ps_o = pmm.tile([128, 512], f32, name=f"wo_ps{p}_{sb}_{n}",
                                                tag="mm", bufs=3)
                                for h in range(HPC):
                                    nc.tensor.matmul(ps_o[:], at_ps[h][:, 128 * sb:128 * (sb + 1)],
                                                     wo_sb(h)[:, 512 * n:512 * (n + 1)],
                                                     start=(h == 0), stop=(h == HPC - 1))
                                if sb % 2 == 0:
                                    nc.vector.tensor_copy(ev[:, 512 * n:512 * (n + 1)], ps_o[:])
                                else:
                                    nc.scalar.activation(ev[:, 512 * n:512 * (n + 1)], ps_o[:],
                                                         AF.Copy)
                            nc.scalar.dma_start(out=a2a_in[p][128 * sb:128 * (sb + 1), :],
                                                in_=ev[:])
                        nc.gpsimd.collective_compute(
                            "AllToAll", ALU.bypass,
                            replica_groups=[list(range(NC_))],
                            ins=[a2a_in[p][:]], outs=[a2a_out[p][:]],
                        )
                        if i > 1:
                            reduce_panel(PLIST[i - 1])

                    # held (heaviest) panel: Wo in two hid halves at the very
                    # end -- its compute overlaps the final exchanges
                    for half in range(2):
                        hid0 = (HID // 2) * half
                        for sb in range(4):
                            ev = tp.tile([128, HID // 2], dt, name=f"wol{half}_{sb}",
                                         tag="wolev", bufs=2)
                            for n in range(2):
                                ps_o = pmm.tile([128, 512], f32, name=f"wol_ps{half}_{sb}_{n}",
                                                tag="mm", bufs=3)
                                for h in range(HPC):
                                    nc.tensor.matmul(
                                        ps_o[:], held_at[h][:, 128 * sb:128 * (sb + 1)],
                                        wo_sb(h)[:, hid0 + 512 * n:hid0 + 512 * (n + 1)],
                                        start=(h == 0), stop=(h == HPC - 1))
                                if sb % 2 == 0:
                                    nc.vector.tensor_copy(ev[:, 512 * n:512 * (n + 1)], ps_o[:])
                                else:
                                    nc.scalar.activation(ev[:, 512 * n:512 * (n + 1)],
                                                         ps_o[:], AF.Copy)
                            nc.scalar.dma_start(out=a2a_lin[half][128 * sb:128 * (sb + 1), :],
                                                in_=ev[:])
                        nc.gpsimd.collective_compute(
                            "AllToAll", ALU.bypass,
                            replica_groups=[list(range(NC_))],
                            ins=[a2a_lin[half][:]], outs=[a2a_lout[half][:]],
                        )
                        if half == 0:
                            reduce_panel(PLIST[-1])
                    for half in range(2):
                        hid0 = (HID // 2) * half
                        rbs = []
                        for r in range(NC_):
                            t = redp.tile([SHARD, HID // 2], dt, name=f"rbl{half}_{r}",
                                          tag="rbl", bufs=4)
                            nc.sync.dma_start(out=t[:],
                                              in_=a2a_lout[half][SHARD * r:SHARD * (r + 1), :])
                            rbs.append(t)
                        reduce_rows(rbs,
                                    out_loc[SHARD * HELD:SHARD * (HELD + 1), hid0:hid0 + HID // 2],
                                    f"l{half}")

    nc.compile()
    return nc


def _to_dt(a, dt):
    if dt == bf16:
        return np.ascontiguousarray(a.astype(ml_dtypes.bfloat16))
    return np.ascontiguousarray(a.astype(np.float32))


def _prepare_inputs(dt, hidden_states, position_ids, Wqa, qa_ln_w, Wqb, Wkva, kv_ln_w, Wkvb, Wo):
    perm = np.concatenate([np.arange(0, ROPE, 2), np.arange(1, ROPE, 2)])
    X = np.asarray(hidden_states, np.float32).reshape(S, HID)
    pos_f = np.ascontiguousarray(np.asarray(position_ids, np.float32).reshape(1, S))
    Wqa = np.asarray(Wqa, np.float32)
    Wkva = np.asarray(Wkva, np.float32)
    wa_kv = np.concatenate([Wkva[:, :KVLR], Wkva[:, KVLR:][:, perm]], axis=1)  # (2048, 576)
    wqb_base = np.asarray(Wqb, np.float32) * np.asarray(qa_ln_w, np.float32)[:, None]
    wkvb_base = np.asarray(Wkvb, np.float32) * np.asarray(kv_ln_w, np.float32)[:, None]
    Wo = np.asarray(Wo, np.float32)

    head_blocks = []
    for h in range(NH):
        cols = wqb_base[:, 192 * h:192 * (h + 1)] * SM_SCALE
        nope = cols[:, :NOPE]
        pe_d = cols[:, NOPE:][:, perm]
        rot = np.concatenate([-pe_d[:, 32:], pe_d[:, :32]], axis=1)
        head_blocks.append(np.concatenate([nope, pe_d, rot], axis=1))  # (1536, 256)
    k_blocks = [wkvb_base[:, 256 * h:256 * h + NOPE] for h in range(NH)]
    v_blocks = [wkvb_base[:, 256 * h + NOPE:256 * (h + 1)] for h in range(NH)]

    inv = (1.0 / (THETA ** (np.arange(0, ROPE, 2, dtype=np.float32) / ROPE))).astype(np.float32)
    invf_np = np.concatenate([inv, inv])

    # diagonal masks: block j, mask[r, col] = NEG where col < 128 j + r
    colsi = np.arange(PANEL)[None, :]
    rowsi = np.arange(128)[:, None]
    mask_np = np.concatenate(
        [np.where(colsi < 128 * j + rowsi, NEG, 0.0) for j in range(4)], axis=1
    ).astype(np.float32)

    def pack_rows(w):
        ch = w.shape[0] // 128
        return np.concatenate([w[128 * k:128 * (k + 1), :] for k in range(ch)], axis=1)

    wa_kv_d = _to_dt(pack_rows(wa_kv), dt)
    wa_q_d = _to_dt(pack_rows(Wqa), dt)
    mask_d = _to_dt(mask_np, dt)
    ones_col_d = _to_dt(np.ones((128, 1), np.float32), dt)

    in_maps = []
    for c in range(NC_):
        rows_c = slice(SL * c, SL * (c + 1))
        wqb_c = np.concatenate([head_blocks[HPC * c + h] for h in range(HPC)], axis=1)
        wkk_c = np.concatenate([k_blocks[HPC * c + h] for h in range(HPC)], axis=1)
        wkv_c = np.concatenate([v_blocks[HPC * c + h] for h in range(HPC)], axis=1)
        wo_c = np.concatenate([Wo[VD * (HPC * c + h):VD * (HPC * c + h + 1), :]
                               for h in range(HPC)], axis=0)
        in_maps.append({
            "x_p": _to_dt(pack_rows(np.ascontiguousarray(X[rows_c, :].T)), dt),
            "pos": np.ascontiguousarray(pos_f[:, rows_c]),
            "pos_all": pos_f,
            "wakv_p": wa_kv_d,
            "waq_p": wa_q_d,
            "wqb_p": _to_dt(pack_rows(wqb_c), dt),
            "wkk_p": _to_dt(pack_rows(wkk_c), dt),
            "wkv_p": _to_dt(pack_rows(wkv_c), dt),
            "wo_p": _to_dt(pack_rows(wo_c), dt),
            "mask": mask_d,
            "ones_col": ones_col_d,
            "ones_row": np.ones((1, 128), np.float32),
            "invf_col": invf_np.reshape(ROPE, 1).copy(),
        })
    return in_maps


def run(inputs, trace=False, trace_cores=None, dt=None):
    dt = dt if dt is not None else DT
    key = ("nc", str(dt))
    if key not in _CACHE:
        _CACHE[key] = build_program(dt)
    nc = _CACHE[key]
    in_maps = _prepare_inputs(dt, **inputs)
    res = run_bass_kernel_spmd(nc, in_maps, list(range(NC_)), trace=trace,
                               trace_cores=trace_cores)
    # reassemble: panel p, core c holds global seq rows [512 p + 64 c, 512 p + 64 (c+1))
    out = np.empty((S, HID), np.float32)
    for c in range(NC_):
        o = res.results[c]["out_loc"]
        for p in range(NPANEL):
            out[PANEL * p + SHARD * c:PANEL * p + SHARD * (c + 1), :] = \
                o[SHARD * p:SHARD * (p + 1), :]
    return out.reshape(1, S, HID), res


def kernel(**inputs) -> np.ndarray:
    out, _ = run(inputs, trace=False)
    return out


# revision 32
# speedup vs baseline: 1.1826x; 1.1826x over previous
"""DeepseekV3 MLA flash-attention prefill kernel for 8 Trainium2 NeuronCores.

Sharding strategy (SPMD, one program for all 8 cores):
  Stage A (sequence-parallel, feature-major): core c computes the low-rank
    down-projections q_a = rms_norm(X @ Wqa), c_kv = rms_norm(ckv[:, :512]),
    k_pe(roped) for its 256 rows directly in transposed layout
    (lhsT = weight chunks, rhs = X^T), then AllGathers them (kv first so
    stage B K/V work can overlap the q gather).
  Stage B (head-parallel): core c owns heads {2c, 2c+1}: all q projections
    (Wqb + RoPE) are precomputed per panel, then causal attention runs in
    (k, q) layout: softmax without max-subtraction, fully-masked k-blocks
    skipped, diagonal blocks masked with GpSimd affine_select, per-q
    normalization folded into the attn^T eviction.  Each head's attn^T is
    exchanged with its own AllToAll so the first overlaps the second head.
  Each core then computes its 256 output rows against the full Wo
    (weights preloaded into a disjoint SBUF region early). Host concatenates.
"""

import sys

if '/opt/trn_rl_repo' not in sys.path:
    sys.path.insert(0, '/opt/trn_rl_repo')

import numpy as np
import ml_dtypes

import concourse.bass as bass
import concourse.mybir as mybir
import concourse.tile as tile
from concourse import bacc
from concourse.bass_utils import run_bass_kernel_spmd

f32 = mybir.dt.float32
f32r = mybir.dt.float32r
bf16 = mybir.dt.bfloat16
i32 = mybir.dt.int32
AF = mybir.ActivationFunctionType
ALU = mybir.AluOpType

NC_ = 8            # cores
S = 2048           # sequence
HID = 2048
QLR = 1536         # q lora rank
KVLR = 512         # kv lora rank
ROPE = 64
NOPE = 128
VD = 128
NH = 16
HPC = NH // NC_    # heads per core = 2
SL = S // NC_      # rows per core = 256
PANEL = 512        # q panel width
NPANEL = S // PANEL
NKB = S // 128     # 16 k blocks
QCH = QLR // 128   # 12
KCH = KVLR // 128  # 4
HCH = HID // 128   # 16
THETA = 10000.0
SM_SCALE = float((NOPE + ROPE) ** -0.5)
PI = float(np.pi)

DT = bf16          # matmul dtype: bf16 or f32r

_CACHE = {}


def _range_reduce_sin(nc, pool, src_ap, P, W, bias, name, tag):
    """sin(src + bias) with range reduction to [-pi, pi]. src may be PSUM."""
    t0 = pool.tile([P, W], f32, name=f"{name}_t0", tag="rr0", bufs=1)
    ti = pool.tile([P, W], i32, name=f"{name}_ti", tag="rr1", bufs=1)
    tf = pool.tile([P, W], f32, name=f"{name}_tf", tag="rr2", bufs=1)
    arg = pool.tile([P, W], f32, name=f"{name}_arg", tag="rr3", bufs=1)
    res = pool.tile([P, W], f32, name=f"{name}_sin", tag=tag, bufs=2)
    nc.vector.tensor_scalar(out=t0[:], in0=src_ap, scalar1=bias, scalar2=None, op0=ALU.add)
    nc.vector.tensor_scalar(out=tf[:], in0=t0[:], scalar1=1.0 / (2 * PI), scalar2=None, op0=ALU.mult)
    nc.vector.tensor_copy(ti[:], tf[:])
    nc.vector.tensor_copy(tf[:], ti[:])
    nc.vector.scalar_tensor_tensor(out=arg[:], in0=tf[:], scalar=-2 * PI, in1=t0[:], op0=ALU.mult, op1=ALU.add)
    nc.scalar.activation(res[:], arg[:], AF.Sin)
    return res


def build_program(dt):
    nc = bacc.Bacc("TRN2", target_bir_lowering=False, debug=False, num_devices=NC_)

    def din(name, shape):
        return nc.dram_tensor(name, shape, dt, kind="ExternalInput")

    # ---- external I/O (per-core data) ----
    x_t = din("x_t", [HID, SL])                 # X rows, transposed (hid-major)
    pos = nc.dram_tensor("pos", [1, SL], f32, kind="ExternalInput")
    pos_all = nc.dram_tensor("pos_all", [1, S], f32, kind="ExternalInput")
    wa = din("wa", [HID, QLR + KVLR + ROPE])    # [Wqa | Wkva(kv) | Wkva(pe, deint)]
    wqb = din("wqb", [QLR, HPC * 256])          # [nope|pe_d|rot] per head
    wkvb_k = din("wkvb_k", [KVLR, HPC * NOPE])
    wkvb_v = din("wkvb_v", [KVLR, HPC * VD])
    wo = din("wo", [NH * VD, HID])
    ones_col = din("ones_col", [128, 1])
    ones_row = nc.dram_tensor("ones_row", [1, 128], f32, kind="ExternalInput")
    invf_col = nc.dram_tensor("invf_col", [ROPE, 1], f32, kind="ExternalInput")
    out_loc = nc.dram_tensor("out_loc", [SL, HID], f32, kind="ExternalOutput")

    NAG_KV = KCH + 1
    WAW = QLR + KVLR + ROPE  # 2112

    with tile.TileContext(nc) as tc:
        with tc.tile_pool(name="dram", bufs=1, space="DRAM") as dpool, \
             tc.tile_pool(name="consts", bufs=1) as cpool:
            ag_in_kv = dpool.tile([NAG_KV * 128, SL], dt)
            ag_out_kv = dpool.tile([NC_ * NAG_KV * 128, SL], dt, addr_space="Shared")
            ag_in_q = dpool.tile([QCH * 128, SL], dt)
            ag_out_q = dpool.tile([NC_ * QCH * 128, SL], dt, addr_space="Shared")
            a2a_in = [dpool.tile([NC_ * VD, SL], dt, name=f"a2a_in{h}") for h in range(HPC)]
            a2a_out = [dpool.tile([NC_ * VD, SL], dt, name=f"a2a_out{h}") for h in range(HPC)]

            ocol = cpool.tile([128, 1], dt)
            orow = cpool.tile([1, 128], f32r)
            invc_t = cpool.tile([ROPE, 1], f32)
            pos_all_t = cpool.tile([1, S], f32r)
            pos_t = cpool.tile([1, SL], f32r)
            nc.sync.dma_start(out=ocol[:], in_=ones_col[:])
            nc.sync.dma_start(out=orow[:], in_=ones_row[:].bitcast(f32r))
            nc.sync.dma_start(out=invc_t[:], in_=invf_col[:])
            nc.sync.dma_start(out=pos_all_t[:], in_=pos_all[:].bitcast(f32r))
            nc.sync.dma_start(out=pos_t[:], in_=pos[:].bitcast(f32r))

            wo_res = False
            wo_map = {}

            # ================= Stage A: transposed down projections =================
            with tc.tile_pool(name="sa_x", bufs=1) as xp, \
                 tc.tile_pool(name="sa_w", bufs=1) as wp, \
                 tc.tile_pool(name="sa_res", bufs=1) as rp, \
                 tc.tile_pool(name="sa_tmp", bufs=2) as tp, \
                 tc.tile_pool(name="sa_ps", bufs=2, space="PSUM") as pp, \
                 tc.tile_pool(name="sa_ps1", bufs=1, space="PSUM") as pp1:

                xts = []
                for k in range(HCH):
                    xt = xp.tile([128, SL], dt, name=f"xt{k}")
                    nc.sync.dma_start(out=xt[:], in_=x_t[128 * k:128 * (k + 1), :])
                    xts.append(xt)
                wa_res = []
                for hc in range(HCH):
                    wt = wp.tile([128, WAW], dt, name=f"wA_{hc}")
                    nc.sync.dma_start(out=wt[:], in_=wa[128 * hc:128 * (hc + 1), :])
                    wa_res.append(wt)

                def a_chunk(o, c0, width, tag):
                    """accumulate chunk [c0:c0+width] of the 2112-wide projection"""
                    ps = pp.tile([width, SL], f32, name=f"ps_{tag}_{o}", tag="a_ps", bufs=2)
                    for hc in range(HCH):
                        nc.tensor.matmul(ps[:], wa_res[hc][:, c0:c0 + width], xts[hc][:],
                                         start=(hc == 0), stop=(hc == HCH - 1))
                    return ps

                ssq_kv = pp1.tile([1, SL], f32, name="ssq_kv")
                kv_sb = []
                for o in range(KCH):
                    ps = a_chunk(o, QLR + 128 * o, 128, "kv")
                    sb = rp.tile([128, SL], f32, name=f"kv_sb{o}")
                    nc.vector.tensor_copy(sb[:], ps[:])
                    kv_sb.append(sb)
                    sq = tp.tile([128, SL], dt, name=f"sqk{o}", tag="sq", bufs=2)
                    nc.scalar.activation(sq[:], ps[:], AF.Square)
                    nc.tensor.matmul(ssq_kv[:], ocol[:], sq[:], start=(o == 0), stop=(o == KCH - 1))
                ps_pe = a_chunk(0, QLR + KVLR, ROPE, "pe")

                # k_pe rope (transposed layout, exact f32 tables)
                tb = pp1.tile([ROPE, SL], f32, name="tb_pe")
                nc.tensor.matmul(tb[:], orow[0:1, 0:ROPE], pos_t[:], start=True, stop=True)
                emb = tp.tile([ROPE, SL], f32, name="emb_pe", tag="emb", bufs=1)
                nc.vector.tensor_scalar(out=emb[:], in0=tb[:], scalar1=invc_t[:], scalar2=None, op0=ALU.mult)
                sin_t = _range_reduce_sin(nc, tp, emb[:], ROPE, SL, 0.0, "sa_s", "sin_s")
                cos_t = _range_reduce_sin(nc, tp, emb[:], ROPE, SL, PI / 2, "sa_c", "sin_c")
                krot = tp.tile([ROPE, SL], f32, name="krot", tag="krot", bufs=1)
                nc.vector.tensor_scalar(out=krot[0:32, :], in0=ps_pe[32:64, :], scalar1=-1.0, scalar2=None, op0=ALU.mult)
                nc.vector.tensor_copy(krot[32:64, :], ps_pe[0:32, :])
                kro = tp.tile([ROPE, SL], f32, name="kro", tag="kro", bufs=1)
                nc.vector.tensor_mul(kro[:], ps_pe[:], cos_t[:])
                krs = tp.tile([ROPE, SL], f32, name="krs", tag="krs", bufs=1)
                nc.vector.tensor_mul(krs[:], krot[:], sin_t[:])
                kfin = tp.tile([ROPE, SL], dt, name="kfin", tag="kfin", bufs=1)
                nc.vector.tensor_add(kfin[:], kro[:], krs[:])
                nc.scalar.dma_start(out=ag_in_kv[KCH * 128:KCH * 128 + ROPE, :], in_=kfin[:])

                # kv rms scale + store
                ms_kv = tp.tile([1, SL], f32, name="ms_kv", tag="ms", bufs=2)
                nc.scalar.activation(ms_kv[:], ssq_kv[:], AF.Sqrt, scale=1.0 / KVLR)
                rkvf = tp.tile([1, SL], f32, name="rkvf", tag="rrf", bufs=2)
                nc.vector.reciprocal_approx_fast(out=rkvf[:], in_=ms_kv[:])
                rkv = tp.tile([1, SL], f32r, name="rkv", tag="rr", bufs=2)
                with nc.allow_low_precision(reason="f32r rounding of rms scale"):
                    nc.vector.tensor_copy(rkv[:], rkvf[:])
                bc_kv = pp1.tile([128, SL], f32, name="bc_kv")
                nc.tensor.matmul(bc_kv[:], orow[:], rkv[:], start=True, stop=True)
                for o in range(KCH):
                    sc = tp.tile([128, SL], dt, name=f"sck{o}", tag="sc", bufs=3)
                    nc.vector.tensor_mul(sc[:], kv_sb[o][:], bc_kv[:])
                    nc.scalar.dma_start(out=ag_in_kv[128 * o:128 * (o + 1), :], in_=sc[:])

                nc.gpsimd.collective_compute(
                    "AllGather", ALU.bypass,
                    replica_groups=[list(range(NC_))],
                    ins=[ag_in_kv[:]], outs=[ag_out_kv[:]],
                )

                # q chunks
                ssq_q = pp1.tile([1, SL], f32, name="ssq_q")
                qa_sb = []
                for o in range(QCH):
                    ps = a_chunk(o, 128 * o, 128, "q")
                    sb = rp.tile([128, SL], f32, name=f"qa_sb{o}")
                    nc.vector.tensor_copy(sb[:], ps[:])
                    qa_sb.append(sb)
                    sq = tp.tile([128, SL], dt, name=f"sqq{o}", tag="sq", bufs=2)
                    nc.scalar.activation(sq[:], ps[:], AF.Square)
                    nc.tensor.matmul(ssq_q[:], ocol[:], sq[:], start=(o == 0), stop=(o == QCH - 1))
                ms_q = tp.tile([1, SL], f32, name="ms_q", tag="ms", bufs=2)
                nc.scalar.activation(ms_q[:], ssq_q[:], AF.Sqrt, scale=1.0 / QLR)
                rqf = tp.tile([1, SL], f32, name="rqf", tag="rrf", bufs=2)
                nc.vector.reciprocal_approx_fast(out=rqf[:], in_=ms_q[:])
                rq = tp.tile([1, SL], f32r, name="rq", tag="rr", bufs=2)
                with nc.allow_low_precision(reason="f32r rounding of rms scale"):
                    nc.vector.tensor_copy(rq[:], rqf[:])
                bc_q = pp1.tile([128, SL], f32, name="bc_q")
                nc.tensor.matmul(bc_q[:], orow[:], rq[:], start=True, stop=True)
                for o in range(QCH):
                    sc = tp.tile([128, SL], dt, name=f"scq{o}", tag="sc", bufs=3)
                    nc.vector.tensor_mul(sc[:], qa_sb[o][:], bc_q[:])
                    nc.scalar.dma_start(out=ag_in_q[128 * o:128 * (o + 1), :], in_=sc[:])

                nc.gpsimd.collective_compute(
                    "AllGather", ALU.bypass,
                    replica_groups=[list(range(NC_))],
                    ins=[ag_in_q[:]], outs=[ag_out_q[:]],
                )

            agkv_r = ag_out_kv.rearrange("(r c) q -> r c q", r=NC_)
            agq_r = ag_out_q.rearrange("(r c) q -> r c q", r=NC_)

            # ================= Stage B: head-parallel attention =================
            with tc.tile_pool(name="sb_res", bufs=1) as rp, \
                 tc.tile_pool(name="sb_qa", bufs=2) as qap, \
                 tc.tile_pool(name="sb_tmp", bufs=2) as tp, \
                 tc.tile_pool(name="sb_pt", bufs=4) as ptp, \
                 tc.tile_pool(name="sb_wo", bufs=1) as wsp, \
                 tc.tile_pool(name="sb_ag", bufs=1) as agp, \
                 tc.tile_pool(name="sb_ps", bufs=2, space="PSUM") as pp, \
                 tc.tile_pool(name="sb_ps1", bufs=1, space="PSUM") as pp1:
                for col in range(HID // 512):
                    for c in range(HCH):
                        t = wsp.tile([128, 512], dt, name=f"wo_s{c}_{col}", tag="wo_s", bufs=40)
                        nc.sync.dma_start(out=t[:], in_=wo[128 * c:128 * (c + 1), 512 * col:512 * (col + 1)])
                        wo_map[(c, col)] = t[:]
                att_g = {}

                kpe_g = rp.tile([ROPE, S], dt, name="kpe_g")
                for r in range(NC_):
                    nc.sync.dma_start(out=kpe_g[:, SL * r:SL * (r + 1)],
                                      in_=agkv_r[r, KCH * 128:KCH * 128 + ROPE, :])

                wqb_t = []
                for l in range(QCH):
                    t = rp.tile([128, HPC * 256], dt, name=f"wqb_t{l}")
                    nc.sync.dma_start(out=t[:], in_=wqb[128 * l:128 * (l + 1), :])
                    wqb_t.append(t)
                wkk_t = []
                wkv_t = []
                for l in range(KCH):
                    t = rp.tile([128, HPC * NOPE], dt, name=f"wkk_t{l}")
                    nc.sync.dma_start(out=t[:], in_=wkvb_k[128 * l:128 * (l + 1), :])
                    wkk_t.append(t)
                    t2 = rp.tile([128, HPC * VD], dt, name=f"wkv_t{l}")
                    nc.sync.dma_start(out=t2[:], in_=wkvb_v[128 * l:128 * (l + 1), :])
                    wkv_t.append(t2)

                # K^T and V (both heads); ckv_g freed afterwards
                kT = [rp.tile([128, S], dt, name=f"kT{h}") for h in range(HPC)]
                v_t = [rp.tile([128, HPC * VD], dt, name=f"v_t{kb}") for kb in range(NKB)]
                with tc.tile_pool(name="sb_ckv", bufs=1) as ckvp:
                    ckv_g = []
                    for j in range(KCH):
                        t = ckvp.tile([128, S], dt, name=f"ckv_g{j}")
                        for r in range(NC_):
                            nc.sync.dma_start(out=t[:, SL * r:SL * (r + 1)],
                                              in_=agkv_r[r, 128 * j:128 * (j + 1), :])
                        ckv_g.append(t)
                    for h in range(HPC):
                        for kc in range(S // 512):
                            ps = pp.tile([128, 512], f32, name=f"kt_ps{h}_{kc}", tag="mm_ps", bufs=2)
                            for l in range(KCH):
                                nc.tensor.matmul(ps[:], wkk_t[l][:, NOPE * h:NOPE * (h + 1)],
                                                 ckv_g[l][:, 512 * kc:512 * (kc + 1)],
                                                 start=(l == 0), stop=(l == KCH - 1))
                            nc.vector.tensor_copy(kT[h][:, 512 * kc:512 * (kc + 1)], ps[:])
                    for kb in range(NKB):
                        ps = pp.tile([128, HPC * VD], f32, name=f"v_ps{kb}", tag="mm_ps", bufs=2)
                        for l in range(KCH):
                            nc.tensor.matmul(ps[:], ckv_g[l][:, 128 * kb:128 * (kb + 1)], wkv_t[l][:],
                                             start=(l == 0), stop=(l == KCH - 1))
                        nc.vector.tensor_copy(v_t[kb][:], ps[:])

                # ---- precompute all q projections (qn + roped qp), panel-major ----
                qn_sb = {}
                qp_sb = {}
                for p in range(NPANEL):
                    qs = slice(PANEL * p, PANEL * (p + 1))
                    qa_p = []
                    for l in range(QCH):
                        t = qap.tile([128, PANEL], dt, name=f"qa_p{p}_{l}", tag=f"qa_p{l}", bufs=2)
                        for r in range(2):
                            nc.sync.dma_start(out=t[:, SL * r:SL * (r + 1)],
                                              in_=agq_r[2 * p + r, 128 * l:128 * (l + 1), :])
                        qa_p.append(t)
                    tb = pp1.tile([ROPE, PANEL], f32, name=f"tbp{p}", tag="bc_ps", bufs=1)
                    nc.tensor.matmul(tb[:], orow[0:1, 0:ROPE], pos_all_t[:, qs], start=True, stop=True)
                    embp = tp.tile([ROPE, PANEL], f32, name=f"embp{p}", tag="embp", bufs=2)
                    nc.vector.tensor_scalar(out=embp[:], in0=tb[:], scalar1=invc_t[:], scalar2=None, op0=ALU.mult)
                    sin_p = _range_reduce_sin(nc, tp, embp[:], ROPE, PANEL, 0.0, f"sb_s{p}", "sin_s")
                    cos_p = _range_reduce_sin(nc, tp, embp[:], ROPE, PANEL, PI / 2, f"sb_c{p}", "sin_c")
                    for h in range(HPC):
                        hcol = 256 * h
                        ps_qn = pp.tile([128, PANEL], f32, name=f"qn_ps{h}_{p}", tag="mm_ps", bufs=2)
                        for l in range(QCH):
                            nc.tensor.matmul(ps_qn[:], wqb_t[l][:, hcol:hcol + NOPE], qa_p[l][:],
                                             start=(l == 0), stop=(l == QCH - 1))
                        ps_qr = pp.tile([128, PANEL], f32, name=f"qr_ps{h}_{p}", tag="mm_ps", bufs=2)
                        for l in range(QCH):
                            nc.tensor.matmul(ps_qr[:], wqb_t[l][:, hcol + NOPE:hcol + 256], qa_p[l][:],
                                             start=(l == 0), stop=(l == QCH - 1))
                        qn = rp.tile([128, PANEL], dt, name=f"qn_sb{h}_{p}")
                        nc.vector.tensor_copy(qn[:], ps_qn[:])
                        qn_sb[(h, p)] = qn
                        qt1 = tp.tile([ROPE, PANEL], f32, name=f"qt1_{h}_{p}", tag="qt1", bufs=2)
                        nc.vector.tensor_mul(qt1[:], ps_qr[0:ROPE, :], cos_p[:])
                        qt2 = tp.tile([ROPE, PANEL], f32, name=f"qt2_{h}_{p}", tag="qt2", bufs=2)
                        nc.vector.tensor_mul(qt2[:], ps_qr[ROPE:2 * ROPE, :], sin_p[:])
                        qp = rp.tile([ROPE, PANEL], dt, name=f"qp_sb{h}_{p}")
                        nc.vector.tensor_add(qp[:], qt1[:], qt2[:])
                        qp_sb[(h, p)] = qp

                # ---- attention ----
                for h in range(HPC):
                    for p in range(NPANEL):
                        nkb = 4 * (p + 1)
                        ps_at = pp.tile([128, PANEL], f32, name=f"at_ps{h}_{p}", tag="at_ps", bufs=1)
                        ps_sum = pp1.tile([1, PANEL], f32, name=f"sum_ps{h}_{p}", tag="sum_ps", bufs=1)
                        pts = {}

                        def consume(kb):
                            nc.tensor.matmul(ps_sum[:], ocol[:], pts[kb][:],
                                             start=(kb == 0), stop=(kb == nkb - 1))
                            nc.tensor.matmul(ps_at[:], v_t[kb][:, VD * h:VD * (h + 1)], pts[kb][:],
                                             start=(kb == 0), stop=(kb == nkb - 1))

                        for kb in range(nkb):
                            ps_sc = pp.tile([128, PANEL], f32, name=f"sc_ps{h}_{p}_{kb}", tag="sc_ps", bufs=3)
                            nc.tensor.matmul(ps_sc[:], kT[h][:, 128 * kb:128 * (kb + 1)], qn_sb[(h, p)][:],
                                             start=True, stop=False)
                            nc.tensor.matmul(ps_sc[:], kpe_g[:, 128 * kb:128 * (kb + 1)], qp_sb[(h, p)][:],
                                             start=False, stop=True)
                            pt = ptp.tile([128, PANEL], dt, name=f"pt{h}_{p}_{kb}", tag="pt", bufs=4)
                            nc.scalar.activation(pt[:], ps_sc[:], AF.Exp, scale=SM_SCALE)
                            if kb >= 4 * p:
                                j = kb - 4 * p
                                nc.gpsimd.affine_select(
                                    out=pt[:], in_=pt[:],
                                    pattern=[[1, PANEL]],
                                    compare_op=ALU.is_ge,
                                    fill=0.0,
                                    base=-128 * j,
                                    channel_multiplier=-1)
                            pts[kb] = pt
                            if kb > 0:
                                consume(kb - 1)
                        consume(nkb - 1)
                        recf = tp.tile([1, PANEL], f32, name=f"recf{h}_{p}", tag="recf", bufs=2)
                        nc.vector.reciprocal_approx_fast(out=recf[:], in_=ps_sum[:])
                        rec = tp.tile([1, PANEL], f32r, name=f"rec{h}_{p}", tag="rec", bufs=2)
                        with nc.allow_low_precision(reason="f32r rounding of softmax recip"):
                            nc.vector.tensor_copy(rec[:], recf[:])
                        bc = pp1.tile([128, PANEL], f32, name=f"bc_ps{h}_{p}", tag="bc_ps", bufs=1)
                        nc.tensor.matmul(bc[:], orow[:], rec[:], start=True, stop=True)
                        bc_sb = tp.tile([128, PANEL], f32, name=f"bc_sb{h}_{p}", tag="bc_sb", bufs=2)
                        nc.vector.tensor_copy(bc_sb[:], bc[:])
                        at_p = tp.tile([128, PANEL], dt, name=f"at_p{h}_{p}", tag="at_p", bufs=2)
                        nc.vector.tensor_mul(at_p[:], ps_at[:], bc_sb[:])
                        for r in range(2):
                            j = 2 * p + r
                            nc.scalar.dma_start(
                                out=a2a_in[h][j * VD:(j + 1) * VD, :],
                                in_=at_p[:, SL * r:SL * (r + 1)])
                    nc.gpsimd.collective_compute(
                        "AllToAll", ALU.bypass,
                        replica_groups=[list(range(NC_))],
                        ins=[a2a_in[h][:]], outs=[a2a_out[h][:]],
                    )
                    for j in range(NC_):
                        c = 2 * j + h
                        t = agp.tile([128, SL], dt, name=f"att_g{c}")
                        nc.sync.dma_start(out=t[:], in_=a2a_out[h][128 * j:128 * (j + 1), :])
                        att_g[c] = t

                # ---- Wo: seq-parallel output projection ----
                for col in range(HID // 512):
                    for qb in range(SL // 128):
                        ps = pp.tile([128, 512], f32, name=f"o_ps{col}_{qb}", tag="mm_ps", bufs=2)
                        for c in range(HCH):
                            nc.tensor.matmul(ps[:], att_g[c][:, 128 * qb:128 * (qb + 1)], wo_map[(c, col)],
                                             start=(c == 0), stop=(c == HCH - 1))
                        osb = tp.tile([128, 512], f32, name=f"osb{col}_{qb}", tag="osb", bufs=3)
                        nc.vector.tensor_copy(osb[:], ps[:])
                        nc.sync.dma_start(out=out_loc[128 * qb:128 * (qb + 1), 512 * col:512 * (col + 1)], in_=osb[:])

    nc.compile()
    return nc


def _to_dt(a, dt):
    if dt == bf16:
        return np.ascontiguousarray(a.astype(ml_dtypes.bfloat16))
    return np.ascontiguousarray(a.astype(np.float32))


def _prepare_inputs(dt, hidden_states, position_ids, Wqa, qa_ln_w, Wqb, Wkva, kv_ln_w, Wkvb, Wo):
    perm = np.concatenate([np.arange(0, ROPE, 2), np.arange(1, ROPE, 2)])
    X = np.asarray(hidden_states, np.float32).reshape(S, HID)
    pos_f = np.ascontiguousarray(np.asarray(position_ids, np.float32).reshape(1, S))
    Wqa = np.asarray(Wqa, np.float32)
    Wkva = np.asarray(Wkva, np.float32)
    wa = np.concatenate([Wqa, Wkva[:, :KVLR], Wkva[:, KVLR:][:, perm]], axis=1)  # (2048, 2112)
    wqb_base = np.asarray(Wqb, np.float32) * np.asarray(qa_ln_w, np.float32)[:, None]
    wkvb_base = np.asarray(Wkvb, np.float32) * np.asarray(kv_ln_w, np.float32)[:, None]
    Wo = np.asarray(Wo, np.float32)

    head_blocks = []
    for h in range(NH):
        cols = wqb_base[:, 192 * h:192 * (h + 1)]
        nope = cols[:, :NOPE]
        pe_d = cols[:, NOPE:][:, perm]
        rot = np.concatenate([-pe_d[:, 32:], pe_d[:, :32]], axis=1)
        head_blocks.append(np.concatenate([nope, pe_d, rot], axis=1))  # (1536, 256)
    k_blocks = [wkvb_base[:, 256 * h:256 * h + NOPE] for h in range(NH)]
    v_blocks = [wkvb_base[:, 256 * h + NOPE:256 * (h + 1)] for h in range(NH)]

    inv = (1.0 / (THETA ** (np.arange(0, ROPE, 2, dtype=np.float32) / ROPE))).astype(np.float32)
    invf_np = np.concatenate([inv, inv])

    wa_d = _to_dt(wa, dt)
    wo_d = _to_dt(Wo, dt)
    ones_col_d = _to_dt(np.ones((128, 1), np.float32), dt)

    in_maps = []
    for c in range(NC_):
        rows = slice(SL * c, SL * (c + 1))
        in_maps.append({
            "x_t": _to_dt(X[rows, :].T, dt),
            "pos": np.ascontiguousarray(pos_f[:, rows]),
            "pos_all": pos_f,
            "wa": wa_d,
            "wqb": _to_dt(np.concatenate([head_blocks[HPC * c + h] for h in range(HPC)], axis=1), dt),
            "wkvb_k": _to_dt(np.concatenate([k_blocks[HPC * c + h] for h in range(HPC)], axis=1), dt),
            "wkvb_v": _to_dt(np.concatenate([v_blocks[HPC * c + h] for h in range(HPC)], axis=1), dt),
            "wo": wo_d,
            "ones_col": ones_col_d,
            "ones_row": np.ones((1, 128), np.float32),
            "invf_col": invf_np.reshape(ROPE, 1).copy(),
        })
    return in_maps


def run(inputs, trace=False, trace_cores=None, dt=None):
    dt = dt if dt is not None else DT
    key = ("nc", str(dt))
    if key not in _CACHE:
        _CACHE[key] = build_program(dt)
    nc = _CACHE[key]
    in_maps = _prepare_inputs(dt, **inputs)
    res = run_bass_kernel_spmd(nc, in_maps, list(range(NC_)), trace=trace,
                               trace_cores=trace_cores)
    out = np.concatenate([res.results[c]["out_loc"] for c in range(NC_)], axis=0)
    return out.reshape(1, S, HID), res


def kernel(**inputs) -> np.ndarray:
    out, _ = run(inputs, trace=False)
    return out

